# revision 17
# baseline (speedup 1.0000x reference)
"""Trainium2 Bass kernel for nn_MemristiveLinear.

The reference's differential-conductance-pair math collapses exactly:
  g_pos - g_neg = k_cond * weights   (the G_OFF leak terms cancel)
so total_currents = K_V * inputs @ (k_cond * weights) and
  y = total_currents / (K_V * k_cond) = inputs @ weights = x @ w + b.

Device kernel: yT = w_shard.T @ x_shardT in bf16 (f32 PSUM accumulate),
sharded over 8 NeuronCores in a 2 (batch) x 4 (n_out) grid; the bias is
added on the host (a [512] vector broadcast, negligible).

Critical-path structure (driven by the DMA fixed costs: ~625ns HWDGE
issue, ~650ns DGE->DMA handoff, ~900ns DMA-completion sem propagation,
and the ~600ns GPSIMD library-load all-engine barrier at kernel start):
 - input [128, 1536] bf16 packed per-partition as
   [w0..w3 | x-slice0 (ko-major) | x-slice1 | x-slice2], split into up
   to 4 chunks whose transfers run back-to-back on the DMA engines
   while matmuls consume already-landed chunks:
     A via SP HWDGE (transfer ready first),
     M via GPSIMD SWDGE dma_start (desc-gen overlaps A's issue, giving
       a second DMA lane that is ready before ACT's),
     S via a second SP HWDGE (optional),
     C via ACT HWDGE.
   Each matmul is gated on the chunk containing ITS x block.
 - dummy matmuls on scratch SBUF keep the PE p-state ramp warm while
   input streams.
 - per-slice PSUM tensors (separate banks): a PSUM bank must not be
   read while matmuls still accumulate into it (reading mid-accumulate
   wedges the device), so each batch-column slice accumulates in its
   own bank and is copied out while later slices still accumulate.
 - PSUM->SBUF copies per slice split across ACT and DVE.
 - output written by ONE PREPARED kv_writeback + trigger_dma:
   descriptors are generated mid-stream on GPSIMD, so after the last
   copy only the trigger + transfer + completion-sem remain on the
   critical path (a plain HWDGE store would re-pay issue+DGE latency).
"""

import contextlib

import numpy as np
import ml_dtypes

import concourse.bacc as bacc
import concourse.mybir as mybir
from concourse.ap import AP
from concourse.bass_utils import run_bass_kernel_spmd

BF16 = ml_dtypes.bfloat16

N_CORES = 8
B, NIN, NOUT = 512, 512, 512
GB, GN = 2, 4                  # batch groups x n_out groups
BS, NS = B // GB, NOUT // GN   # 256 batch cols, 128 n_out rows per core
P = 128
KO = NIN // P                  # 4 contraction blocks
W_ELEMS = KO * NS              # 512 w elems per partition
INW = W_ELEMS + KO * BS        # 1536 elems per partition

# ---- tunables ----
SLICES = (64, 96, 96)          # batch-column slices (sum = BS)
COPY_ENG = ("dve", "act", "dve")   # PSUM->SBUF copy engine per slice
# chunk boundaries in elems/partition (ascending; equal = empty chunk):
#   A = [0, b0) SP#1, M = [b0, b1) gpsimd SWDGE, S = [b1, b2) SP#2,
#   C = [b2, INW) ACT
BOUNDS = (W_ELEMS + KO * 64, W_ELEMS + KO * 160, W_ELEMS + KO * 160)
WARMUP = 23                    # 128-wide PE warmup matmuls
# filler matmul widths inserted after slice s's matmuls (keeps the PE
# busy across gaps between input-chunk sems): {slice_index: [widths]}
FILLERS = {}

_NC = None


def _cfg(slices, bounds):
    cum = [sum(slices[:i]) for i in range(len(slices) + 1)]

    def x_off(s, k):
        return W_ELEMS + KO * cum[s] + k * slices[s]

    def gate_of(s, k):
        """input chunk index (0=A,1=M,2=S,3=C) containing x block (s,k)"""
        end = x_off(s, k) + slices[s]
        for i, b in enumerate(bounds):
            if end <= b:
                return i
        return 3

    return cum, x_off, gate_of


def _build(n_iters=1, slices=None, copy_eng=None, warmup=None, bounds=None,
           fillers=None):
    assert n_iters == 1
    slices = slices or SLICES
    copy_eng = copy_eng or COPY_ENG
    warmup = WARMUP if warmup is None else warmup
    bounds = BOUNDS if bounds is None else bounds
    fillers = FILLERS if fillers is None else fillers
    nsl = len(slices)
    assert len(bounds) == 3 and list(bounds) == sorted(bounds)
    b0, b1, b2 = bounds
    cum, x_off, gate_of = _cfg(slices, bounds)

    nc = bacc.Bacc("TRN2", target_bir_lowering=False, debug=False,
                   num_devices=N_CORES)
    bf16 = mybir.dt.bfloat16
    f32 = mybir.dt.float32
    i32 = mybir.dt.int32
    inp = nc.dram_tensor("inp", [P, INW], bf16, kind="ExternalInput")
    y = nc.dram_tensor("y", [NS, BS], f32, kind="ExternalOutput")
    with contextlib.ExitStack() as ctx:
        sem = lambda name: ctx.enter_context(nc.semaphore(name))
        s_in = [sem(f"s_in{i}") for i in range(4)]   # chunk completions
        s_scr = sem("s_scr")                 # scratch memset done
        s_mm = sem("s_mm")                   # per-slice matmul completion
        s_cp = [sem(f"s_cp{i}") for i in range(nsl)]   # copy done
        s_pk = sem("s_pk")                   # kv prep (EVSEM)
        s_out = sem("s_out")                 # writeback done

        t_in = ctx.enter_context(nc.sbuf_tensor("t_in", [P, INW], bf16))
        t_out = ctx.enter_context(nc.sbuf_tensor("t_out", [NS, BS], f32))
        t_scr = ctx.enter_context(nc.sbuf_tensor("t_scr", [P, P], bf16))
        t_ctx = ctx.enter_context(nc.sbuf_tensor("t_ctx", [P, 1], i32))

        # one PSUM bank per slice: a bank must not be read while
        # matmuls still accumulate into it
        pss = [nc.alloc_psum_tensor(f"ps{s}", [NS, slices[s]], f32)
               for s in range(nsl)]
        ps_d = nc.alloc_psum_tensor("ps_d", [P, P], f32)

        # ---- input chunks ----
        nc.sync.dma_start(t_in[:, 0:b0],
                          inp.ap()[:, 0:b0]).then_inc(s_in[0], 16)
        if b2 > b1:
            nc.sync.dma_start(t_in[:, b1:b2],
                              inp.ap()[:, b1:b2]).then_inc(s_in[2], 16)
        if INW > b2:
            nc.scalar.dma_start(t_in[:, b2:INW],
                                inp.ap()[:, b2:INW]).then_inc(s_in[3], 16)
        if b1 > b0:
            # GPSIMD SWDGE lane: desc-gen overlaps the SP chunk's
            # issue+transfer; ready before the ACT lane
            nc.gpsimd.dma_start(t_in[:, b0:b1],
                                inp.ap()[:, b0:b1]).then_inc(s_in[1], 16)

        # ---- GPSIMD: kv_writeback prep for the output ----
        # ctx indices (all zeros); same-engine order guarantees they're
        # written before the prep reads them
        nc.gpsimd.memset(t_ctx[:, :], 0)
        in4 = AP(t_out[:, :].tensor, 0, [[BS, NS], [BS, 1], [BS, 1], [1, BS]])
        out4 = AP(y.ap().tensor, 0, [[BS * NS, 1], [BS, NS], [BS, 1], [1, BS]])
        nc.gpsimd.kv_writeback(out4, in4, t_ctx[:, 0:1],
                               prepare_only=True,
                               sem=s_out).then_inc(s_pk, 1)
        # waits: early sems first; the LAST copy's wait rides on the
        # trigger itself so its decode happens while parked
        cp_target = [2 if copy_eng[s] == "both" else 1 for s in range(nsl)]
        nc.gpsimd.wait_ge(s_pk, 1)
        for s in range(nsl - 1):
            nc.gpsimd.wait_ge(s_cp[s], cp_target[s])
        nc.gpsimd.trigger_dma(1).wait_op(s_cp[nsl - 1], cp_target[nsl - 1],
                                         "sem-ge")

        # ---- DVE: scratch memset for warmups, then its copies ----
        nc.vector.memset(t_scr[:, :], 0).then_inc(s_scr, 1)

        # ---- PE: warmups + real matmuls (per-ko chunk gating) ----
        nc.tensor.wait_ge(s_scr, 1)
        for _ in range(warmup):
            nc.tensor.matmul(ps_d.ap(), t_scr[:, :], t_scr[:, :],
                             start=True, stop=True)
        seen_gate = -1
        for s in range(nsl):
            w = slices[s]
            mm = None
            for k in range(KO):
                g = gate_of(s, k)
                if g > seen_gate:
                    nc.tensor.wait_ge(s_in[g], 16)
                    seen_gate = g
                mm = nc.tensor.matmul(
                    pss[s].ap(),
                    t_in[:, k * NS:(k + 1) * NS],
                    t_in[:, x_off(s, k):x_off(s, k) + w],
                    start=(k == 0),
                    stop=(k == KO - 1),
                )
            mm.then_inc(s_mm, 1)
            for fw in fillers.get(s, ()):
                nc.tensor.matmul(ps_d.ap()[:, 0:fw], t_scr[:, :],
                                 t_scr[:, 0:fw], start=True, stop=True)

        # ---- PSUM -> SBUF copies (per slice, on copy_eng) ----
        for s in range(nsl):
            w, cb = slices[s], cum[s]
            if copy_eng[s] == "both":
                # split across DVE and ACT in parallel
                dw = (w // 2 + 15) & ~15
                nc.vector.wait_ge(s_mm, s + 1)
                nc.vector.tensor_scalar_mul(
                    t_out[:, cb:cb + dw], pss[s].ap()[:, 0:dw],
                    1.0).then_inc(s_cp[s], 1)
                nc.scalar.wait_ge(s_mm, s + 1)
                nc.scalar.copy(t_out[:, cb + dw:cb + w],
                               pss[s].ap()[:, dw:w]).then_inc(s_cp[s], 1)
            elif copy_eng[s] == "dve":
                nc.vector.wait_ge(s_mm, s + 1)
                nc.vector.tensor_scalar_mul(
                    t_out[:, cb:cb + w], pss[s].ap(),
                    1.0).then_inc(s_cp[s], 1)
            else:
                nc.scalar.wait_ge(s_mm, s + 1)
                nc.scalar.copy(t_out[:, cb:cb + w],
                               pss[s].ap()).then_inc(s_cp[s], 1)

        # ---- drain: NEFF must not end before the output landed ----
        nc.sync.wait_ge(s_out, 16)
    nc.compile()
    return nc


def _get_nc():
    global _NC
    if _NC is None:
        _NC = _build()
    return _NC


def _make_in_maps(x, w, b, slices=None):
    slices = slices or SLICES
    cum = [sum(slices[:i]) for i in range(len(slices) + 1)]
    xb = np.asarray(x, dtype=np.float32).astype(BF16)
    wb = np.asarray(w, dtype=np.float32).astype(BF16)
    xbT = np.ascontiguousarray(xb.T)               # [NIN, B]
    wr = wb.reshape(KO, P, NOUT)                   # [ko, p, nout]
    xr = xbT.reshape(KO, P, B)                     # [ko, p, batch]
    in_maps = []
    for c in range(N_CORES):
        gb, gn = divmod(c, GN)
        pack = np.empty((P, INW), BF16)
        pack[:, 0:W_ELEMS] = (
            wr[:, :, gn * NS:(gn + 1) * NS].transpose(1, 0, 2).reshape(P, W_ELEMS))
        xcore = xr[:, :, gb * BS:(gb + 1) * BS]    # [ko, p, BS]
        for s in range(len(slices)):
            for k in range(KO):
                off = W_ELEMS + KO * cum[s] + k * slices[s]
                pack[:, off:off + slices[s]] = xcore[k, :, cum[s]:cum[s + 1]]
        in_maps.append({"inp": pack})
    return in_maps


def _gather(results, b):
    y = np.empty((B, NOUT), np.float32)
    for c in range(N_CORES):
        gb, gn = divmod(c, GN)
        yt = np.asarray(results[c]["y"]).reshape(NS, BS).astype(np.float32)
        y[gb * BS:(gb + 1) * BS, gn * NS:(gn + 1) * NS] = yt.T
    return y + np.asarray(b, dtype=np.float32)[None, :]


def run(x, w, b, **spmd_kwargs):
    """Run on hardware; returns (y, BassKernelResults)."""
    nc = _get_nc()
    res = run_bass_kernel_spmd(nc, _make_in_maps(x, w, b),
                               list(range(N_CORES)), **spmd_kwargs)
    return _gather(res.results, b), res


def kernel(x, w, b):
    y, _ = run(x, w, b)
    return y


# revision 18
# speedup vs baseline: 1.0030x; 1.0030x over previous
"""Trainium2 Bass kernel for nn_MemristiveLinear.

The reference's differential-conductance-pair math collapses exactly:
  g_pos - g_neg = k_cond * weights   (the G_OFF leak terms cancel)
so total_currents = K_V * inputs @ (k_cond * weights) and
  y = total_currents / (K_V * k_cond) = inputs @ weights = x @ w + b.

Device kernel: yT = w_shard.T @ x_shardT in bf16 (f32 PSUM accumulate),
sharded over 8 NeuronCores in a 2 (batch) x 4 (n_out) grid; the bias is
added on the host (a [512] vector broadcast, negligible).

Critical-path structure (driven by the DMA fixed costs: ~625ns HWDGE
issue, ~650ns DGE->DMA handoff, ~900ns DMA-completion sem propagation,
and the ~600ns GPSIMD library-load all-engine barrier at kernel start):
 - input [128, 1536] bf16 packed per-partition as
   [w0..w3 | x-slice0 (ko-major) | x-slice1 | x-slice2], split into up
   to 4 chunks whose transfers run back-to-back on the DMA engines
   while matmuls consume already-landed chunks:
     A via SP HWDGE (transfer ready first),
     M via GPSIMD SWDGE dma_start (desc-gen overlaps A's issue, giving
       a second DMA lane that is ready before ACT's),
     S via a second SP HWDGE (optional),
     C via ACT HWDGE.
   Each matmul is gated on the chunk containing ITS x block.
 - dummy matmuls on scratch SBUF keep the PE p-state ramp warm while
   input streams.
 - per-slice PSUM tensors (separate banks): a PSUM bank must not be
   read while matmuls still accumulate into it (reading mid-accumulate
   wedges the device), so each batch-column slice accumulates in its
   own bank and is copied out while later slices still accumulate.
 - PSUM->SBUF copies per slice split across ACT and DVE.
 - output written by ONE PREPARED kv_writeback + trigger_dma:
   descriptors are generated mid-stream on GPSIMD, so after the last
   copy only the trigger + transfer + completion-sem remain on the
   critical path (a plain HWDGE store would re-pay issue+DGE latency).
"""

import contextlib

import numpy as np
import ml_dtypes

import concourse.bacc as bacc
import concourse.mybir as mybir
from concourse.ap import AP
from concourse.bass_utils import run_bass_kernel_spmd

BF16 = ml_dtypes.bfloat16

N_CORES = 8
B, NIN, NOUT = 512, 512, 512
GB, GN = 2, 4                  # batch groups x n_out groups
BS, NS = B // GB, NOUT // GN   # 256 batch cols, 128 n_out rows per core
P = 128
KO = NIN // P                  # 4 contraction blocks
W_ELEMS = KO * NS              # 512 w elems per partition
INW = W_ELEMS + KO * BS        # 1536 elems per partition

# ---- tunables ----
SLICES = (64, 104, 88)         # batch-column slices (sum = BS)
COPY_ENG = ("dve", "act", "dve")   # PSUM->SBUF copy engine per slice
# chunk boundaries in elems/partition (ascending; equal = empty chunk):
#   A = [0, b0) SP#1, M = [b0, b1) gpsimd SWDGE, S = [b1, b2) SP#2,
#   C = [b2, INW) ACT
BOUNDS = (W_ELEMS + KO * 64, W_ELEMS + KO * 168, W_ELEMS + KO * 168)
WARMUP = 23                    # 128-wide PE warmup matmuls
# filler matmul widths inserted after slice s's matmuls (keeps the PE
# busy across gaps between input-chunk sems): {slice_index: [widths]}
FILLERS = {}

_NC = None


def _cfg(slices, bounds):
    cum = [sum(slices[:i]) for i in range(len(slices) + 1)]

    def x_off(s, k):
        return W_ELEMS + KO * cum[s] + k * slices[s]

    def gate_of(s, k):
        """input chunk index (0=A,1=M,2=S,3=C) containing x block (s,k)"""
        end = x_off(s, k) + slices[s]
        for i, b in enumerate(bounds):
            if end <= b:
                return i
        return 3

    return cum, x_off, gate_of


def _build(n_iters=1, slices=None, copy_eng=None, warmup=None, bounds=None,
           fillers=None):
    assert n_iters == 1
    slices = slices or SLICES
    copy_eng = copy_eng or COPY_ENG
    warmup = WARMUP if warmup is None else warmup
    bounds = BOUNDS if bounds is None else bounds
    fillers = FILLERS if fillers is None else fillers
    nsl = len(slices)
    assert len(bounds) == 3 and list(bounds) == sorted(bounds)
    b0, b1, b2 = bounds
    cum, x_off, gate_of = _cfg(slices, bounds)

    nc = bacc.Bacc("TRN2", target_bir_lowering=False, debug=False,
                   num_devices=N_CORES)
    bf16 = mybir.dt.bfloat16
    f32 = mybir.dt.float32
    i32 = mybir.dt.int32
    inp = nc.dram_tensor("inp", [P, INW], bf16, kind="ExternalInput")
    y = nc.dram_tensor("y", [NS, BS], f32, kind="ExternalOutput")
    with contextlib.ExitStack() as ctx:
        sem = lambda name: ctx.enter_context(nc.semaphore(name))
        s_in = [sem(f"s_in{i}") for i in range(4)]   # chunk completions
        s_scr = sem("s_scr")                 # scratch memset done
        s_mm = sem("s_mm")                   # per-slice matmul completion
        s_cp = [sem(f"s_cp{i}") for i in range(nsl)]   # copy done
        s_pk = sem("s_pk")                   # kv prep (EVSEM)
        s_out = sem("s_out")                 # writeback done

        t_in = ctx.enter_context(nc.sbuf_tensor("t_in", [P, INW], bf16))
        t_out = ctx.enter_context(nc.sbuf_tensor("t_out", [NS, BS], f32))
        t_scr = ctx.enter_context(nc.sbuf_tensor("t_scr", [P, P], bf16))
        t_ctx = ctx.enter_context(nc.sbuf_tensor("t_ctx", [P, 1], i32))

        # one PSUM bank per slice: a bank must not be read while
        # matmuls still accumulate into it
        pss = [nc.alloc_psum_tensor(f"ps{s}", [NS, slices[s]], f32)
               for s in range(nsl)]
        ps_d = nc.alloc_psum_tensor("ps_d", [P, P], f32)

        # ---- input chunks ----
        nc.sync.dma_start(t_in[:, 0:b0],
                          inp.ap()[:, 0:b0]).then_inc(s_in[0], 16)
        if b2 > b1:
            nc.sync.dma_start(t_in[:, b1:b2],
                              inp.ap()[:, b1:b2]).then_inc(s_in[2], 16)
        if INW > b2:
            nc.scalar.dma_start(t_in[:, b2:INW],
                                inp.ap()[:, b2:INW]).then_inc(s_in[3], 16)
        if b1 > b0:
            # GPSIMD SWDGE lane: desc-gen overlaps the SP chunk's
            # issue+transfer; ready before the ACT lane
            nc.gpsimd.dma_start(t_in[:, b0:b1],
                                inp.ap()[:, b0:b1]).then_inc(s_in[1], 16)

        # ---- GPSIMD: kv_writeback prep for the output ----
        # ctx indices (all zeros); same-engine order guarantees they're
        # written before the prep reads them
        nc.gpsimd.memset(t_ctx[:, :], 0)
        in4 = AP(t_out[:, :].tensor, 0, [[BS, NS], [BS, 1], [BS, 1], [1, BS]])
        out4 = AP(y.ap().tensor, 0, [[BS * NS, 1], [BS, NS], [BS, 1], [1, BS]])
        nc.gpsimd.kv_writeback(out4, in4, t_ctx[:, 0:1],
                               prepare_only=True,
                               sem=s_out).then_inc(s_pk, 1)
        # waits: early sems first; the LAST copy's wait rides on the
        # trigger itself so its decode happens while parked
        cp_target = [2 if copy_eng[s] == "both" else 1 for s in range(nsl)]
        nc.gpsimd.wait_ge(s_pk, 1)
        for s in range(nsl - 1):
            nc.gpsimd.wait_ge(s_cp[s], cp_target[s])
        nc.gpsimd.trigger_dma(1).wait_op(s_cp[nsl - 1], cp_target[nsl - 1],
                                         "sem-ge")

        # ---- DVE: scratch memset for warmups, then its copies ----
        nc.vector.memset(t_scr[:, :], 0).then_inc(s_scr, 1)

        # ---- PE: warmups + real matmuls (per-ko chunk gating) ----
        nc.tensor.wait_ge(s_scr, 1)
        for _ in range(warmup):
            nc.tensor.matmul(ps_d.ap(), t_scr[:, :], t_scr[:, :],
                             start=True, stop=True)
        seen_gate = -1
        for s in range(nsl):
            w = slices[s]
            mm = None
            for k in range(KO):
                g = gate_of(s, k)
                if g > seen_gate:
                    nc.tensor.wait_ge(s_in[g], 16)
                    seen_gate = g
                mm = nc.tensor.matmul(
                    pss[s].ap(),
                    t_in[:, k * NS:(k + 1) * NS],
                    t_in[:, x_off(s, k):x_off(s, k) + w],
                    start=(k == 0),
                    stop=(k == KO - 1),
                )
            mm.then_inc(s_mm, 1)
            for fw in fillers.get(s, ()):
                nc.tensor.matmul(ps_d.ap()[:, 0:fw], t_scr[:, :],
                                 t_scr[:, 0:fw], start=True, stop=True)

        # ---- PSUM -> SBUF copies (per slice, on copy_eng) ----
        for s in range(nsl):
            w, cb = slices[s], cum[s]
            if copy_eng[s] == "both":
                # split across DVE and ACT in parallel
                dw = (w // 2 + 15) & ~15
                nc.vector.wait_ge(s_mm, s + 1)
                nc.vector.tensor_scalar_mul(
                    t_out[:, cb:cb + dw], pss[s].ap()[:, 0:dw],
                    1.0).then_inc(s_cp[s], 1)
                nc.scalar.wait_ge(s_mm, s + 1)
                nc.scalar.copy(t_out[:, cb + dw:cb + w],
                               pss[s].ap()[:, dw:w]).then_inc(s_cp[s], 1)
            elif copy_eng[s] == "dve":
                nc.vector.wait_ge(s_mm, s + 1)
                nc.vector.tensor_scalar_mul(
                    t_out[:, cb:cb + w], pss[s].ap(),
                    1.0).then_inc(s_cp[s], 1)
            else:
                nc.scalar.wait_ge(s_mm, s + 1)
                nc.scalar.copy(t_out[:, cb:cb + w],
                               pss[s].ap()).then_inc(s_cp[s], 1)

        # ---- drain: NEFF must not end before the output landed ----
        nc.sync.wait_ge(s_out, 16)
    nc.compile()
    return nc


def _get_nc():
    global _NC
    if _NC is None:
        _NC = _build()
    return _NC


def _make_in_maps(x, w, b, slices=None):
    slices = slices or SLICES
    cum = [sum(slices[:i]) for i in range(len(slices) + 1)]
    xb = np.asarray(x, dtype=np.float32).astype(BF16)
    wb = np.asarray(w, dtype=np.float32).astype(BF16)
    xbT = np.ascontiguousarray(xb.T)               # [NIN, B]
    wr = wb.reshape(KO, P, NOUT)                   # [ko, p, nout]
    xr = xbT.reshape(KO, P, B)                     # [ko, p, batch]
    in_maps = []
    for c in range(N_CORES):
        gb, gn = divmod(c, GN)
        pack = np.empty((P, INW), BF16)
        pack[:, 0:W_ELEMS] = (
            wr[:, :, gn * NS:(gn + 1) * NS].transpose(1, 0, 2).reshape(P, W_ELEMS))
        xcore = xr[:, :, gb * BS:(gb + 1) * BS]    # [ko, p, BS]
        for s in range(len(slices)):
            for k in range(KO):
                off = W_ELEMS + KO * cum[s] + k * slices[s]
                pack[:, off:off + slices[s]] = xcore[k, :, cum[s]:cum[s + 1]]
        in_maps.append({"inp": pack})
    return in_maps


def _gather(results, b):
    y = np.empty((B, NOUT), np.float32)
    for c in range(N_CORES):
        gb, gn = divmod(c, GN)
        yt = np.asarray(results[c]["y"]).reshape(NS, BS).astype(np.float32)
        y[gb * BS:(gb + 1) * BS, gn * NS:(gn + 1) * NS] = yt.T
    return y + np.asarray(b, dtype=np.float32)[None, :]


def run(x, w, b, **spmd_kwargs):
    """Run on hardware; returns (y, BassKernelResults)."""
    nc = _get_nc()
    res = run_bass_kernel_spmd(nc, _make_in_maps(x, w, b),
                               list(range(N_CORES)), **spmd_kwargs)
    return _gather(res.results, b), res


def kernel(x, w, b):
    y, _ = run(x, w, b)
    return y


# revision 19
# speedup vs baseline: 1.0054x; 1.0023x over previous
"""Trainium2 Bass kernel for nn_MemristiveLinear.

The reference's differential-conductance-pair math collapses exactly:
  g_pos - g_neg = k_cond * weights   (the G_OFF leak terms cancel)
so total_currents = K_V * inputs @ (k_cond * weights) and
  y = total_currents / (K_V * k_cond) = inputs @ weights = x @ w + b.

Device kernel: yT = w_shard.T @ x_shardT in bf16 (f32 PSUM accumulate),
sharded over 8 NeuronCores in a 2 (batch) x 4 (n_out) grid; the bias is
added on the host (a [512] vector broadcast, negligible).

Critical-path structure (driven by the DMA fixed costs: ~625ns HWDGE
issue, ~650ns DGE->DMA handoff, ~900ns DMA-completion sem propagation,
and the ~600ns GPSIMD library-load all-engine barrier at kernel start):
 - input [128, 1536] bf16 packed per-partition as
   [w0..w3 | x-slice0 (ko-major) | x-slice1 | x-slice2], split into up
   to 4 chunks whose transfers run back-to-back on the DMA engines
   while matmuls consume already-landed chunks:
     A via SP HWDGE (transfer ready first),
     M via GPSIMD SWDGE dma_start (desc-gen overlaps A's issue, giving
       a second DMA lane that is ready before ACT's),
     S via a second SP HWDGE (optional),
     C via ACT HWDGE.
   Each matmul is gated on the chunk containing ITS x block.
 - dummy matmuls on scratch SBUF keep the PE p-state ramp warm while
   input streams.
 - per-slice PSUM tensors (separate banks): a PSUM bank must not be
   read while matmuls still accumulate into it (reading mid-accumulate
   wedges the device), so each batch-column slice accumulates in its
   own bank and is copied out while later slices still accumulate.
 - PSUM->SBUF copies per slice split across ACT and DVE.
 - output written by ONE PREPARED kv_writeback + trigger_dma:
   descriptors are generated mid-stream on GPSIMD, so after the last
   copy only the trigger + transfer + completion-sem remain on the
   critical path (a plain HWDGE store would re-pay issue+DGE latency).
"""

import contextlib

import numpy as np
import ml_dtypes

import concourse.bacc as bacc
import concourse.mybir as mybir
from concourse.ap import AP
from concourse.bass_utils import run_bass_kernel_spmd

BF16 = ml_dtypes.bfloat16

N_CORES = 8
B, NIN, NOUT = 512, 512, 512
GB, GN = 2, 4                  # batch groups x n_out groups
BS, NS = B // GB, NOUT // GN   # 256 batch cols, 128 n_out rows per core
P = 128
KO = NIN // P                  # 4 contraction blocks
W_ELEMS = KO * NS              # 512 w elems per partition
INW = W_ELEMS + KO * BS        # 1536 elems per partition

# ---- tunables ----
SLICES = (64, 104, 88)         # batch-column slices (sum = BS)
COPY_ENG = ("dve", "act", "dve")   # PSUM->SBUF copy engine per slice
# chunk boundaries in elems/partition (ascending; equal = empty chunk):
#   A = [0, b0) SP#1, M = [b0, b1) gpsimd SWDGE, S = [b1, b2) SP#2,
#   C = [b2, INW) ACT
BOUNDS = (W_ELEMS + KO * 64, W_ELEMS + KO * 168, W_ELEMS + KO * 168)
WARMUP = 23                    # 128-wide PE warmup matmuls
# filler matmul widths inserted after slice s's matmuls (keeps the PE
# busy across gaps between input-chunk sems): {slice_index: [widths]}
FILLERS = {}

_NC = None


def _cfg(slices, bounds):
    cum = [sum(slices[:i]) for i in range(len(slices) + 1)]

    def x_off(s, k):
        return W_ELEMS + KO * cum[s] + k * slices[s]

    def gate_of(s, k):
        """input chunk index (0=A,1=M,2=S,3=C) containing x block (s,k)"""
        end = x_off(s, k) + slices[s]
        for i, b in enumerate(bounds):
            if end <= b:
                return i
        return 3

    return cum, x_off, gate_of


def _build(n_iters=1, slices=None, copy_eng=None, warmup=None, bounds=None,
           fillers=None):
    assert n_iters == 1
    slices = slices or SLICES
    copy_eng = copy_eng or COPY_ENG
    warmup = WARMUP if warmup is None else warmup
    bounds = BOUNDS if bounds is None else bounds
    fillers = FILLERS if fillers is None else fillers
    nsl = len(slices)
    assert len(bounds) == 3 and list(bounds) == sorted(bounds)
    b0, b1, b2 = bounds
    cum, x_off, gate_of = _cfg(slices, bounds)

    nc = bacc.Bacc("TRN2", target_bir_lowering=False, debug=False,
                   num_devices=N_CORES)
    bf16 = mybir.dt.bfloat16
    f32 = mybir.dt.float32
    i32 = mybir.dt.int32
    inp = nc.dram_tensor("inp", [P, INW], bf16, kind="ExternalInput")
    y = nc.dram_tensor("y", [NS, BS], bf16, kind="ExternalOutput")
    with contextlib.ExitStack() as ctx:
        sem = lambda name: ctx.enter_context(nc.semaphore(name))
        s_in = [sem(f"s_in{i}") for i in range(4)]   # chunk completions
        s_scr = sem("s_scr")                 # scratch memset done
        s_mm = sem("s_mm")                   # per-slice matmul completion
        s_cp = [sem(f"s_cp{i}") for i in range(nsl)]   # copy done
        s_pk = sem("s_pk")                   # kv prep (EVSEM)
        s_out = sem("s_out")                 # writeback done

        t_in = ctx.enter_context(nc.sbuf_tensor("t_in", [P, INW], bf16))
        t_out = ctx.enter_context(nc.sbuf_tensor("t_out", [NS, BS], bf16))
        t_scr = ctx.enter_context(nc.sbuf_tensor("t_scr", [P, P], bf16))
        t_ctx = ctx.enter_context(nc.sbuf_tensor("t_ctx", [P, 1], i32))

        # one PSUM bank per slice: a bank must not be read while
        # matmuls still accumulate into it
        pss = [nc.alloc_psum_tensor(f"ps{s}", [NS, slices[s]], f32)
               for s in range(nsl)]
        ps_d = nc.alloc_psum_tensor("ps_d", [P, P], f32)

        # ---- input chunks ----
        nc.sync.dma_start(t_in[:, 0:b0],
                          inp.ap()[:, 0:b0]).then_inc(s_in[0], 16)
        if b2 > b1:
            nc.sync.dma_start(t_in[:, b1:b2],
                              inp.ap()[:, b1:b2]).then_inc(s_in[2], 16)
        if INW > b2:
            nc.scalar.dma_start(t_in[:, b2:INW],
                                inp.ap()[:, b2:INW]).then_inc(s_in[3], 16)
        if b1 > b0:
            # GPSIMD SWDGE lane: desc-gen overlaps the SP chunk's
            # issue+transfer; ready before the ACT lane
            nc.gpsimd.dma_start(t_in[:, b0:b1],
                                inp.ap()[:, b0:b1]).then_inc(s_in[1], 16)

        # ---- GPSIMD: kv_writeback prep for the output ----
        # ctx indices (all zeros); same-engine order guarantees they're
        # written before the prep reads them
        nc.gpsimd.memset(t_ctx[:, :], 0)
        in4 = AP(t_out[:, :].tensor, 0, [[BS, NS], [BS, 1], [BS, 1], [1, BS]])
        out4 = AP(y.ap().tensor, 0, [[BS * NS, 1], [BS, NS], [BS, 1], [1, BS]])
        nc.gpsimd.kv_writeback(out4, in4, t_ctx[:, 0:1],
                               prepare_only=True,
                               sem=s_out).then_inc(s_pk, 1)
        # waits: early sems first; the LAST copy's wait rides on the
        # trigger itself so its decode happens while parked
        cp_target = [2 if copy_eng[s] == "both" else 1 for s in range(nsl)]
        nc.gpsimd.wait_ge(s_pk, 1)
        for s in range(nsl - 1):
            nc.gpsimd.wait_ge(s_cp[s], cp_target[s])
        nc.gpsimd.trigger_dma(1).wait_op(s_cp[nsl - 1], cp_target[nsl - 1],
                                         "sem-ge")

        # ---- DVE: scratch memset for warmups, then its copies ----
        nc.vector.memset(t_scr[:, :], 0).then_inc(s_scr, 1)

        # ---- PE: warmups + real matmuls (per-ko chunk gating) ----
        nc.tensor.wait_ge(s_scr, 1)
        for _ in range(warmup):
            nc.tensor.matmul(ps_d.ap(), t_scr[:, :], t_scr[:, :],
                             start=True, stop=True)
        seen_gate = -1
        for s in range(nsl):
            w = slices[s]
            mm = None
            for k in range(KO):
                g = gate_of(s, k)
                if g > seen_gate:
                    nc.tensor.wait_ge(s_in[g], 16)
                    seen_gate = g
                mm = nc.tensor.matmul(
                    pss[s].ap(),
                    t_in[:, k * NS:(k + 1) * NS],
                    t_in[:, x_off(s, k):x_off(s, k) + w],
                    start=(k == 0),
                    stop=(k == KO - 1),
                )
            mm.then_inc(s_mm, 1)
            for fw in fillers.get(s, ()):
                nc.tensor.matmul(ps_d.ap()[:, 0:fw], t_scr[:, :],
                                 t_scr[:, 0:fw], start=True, stop=True)

        # ---- PSUM -> SBUF copies (per slice, on copy_eng) ----
        for s in range(nsl):
            w, cb = slices[s], cum[s]
            if copy_eng[s] == "both":
                # split across DVE and ACT in parallel
                dw = (w // 2 + 15) & ~15
                nc.vector.wait_ge(s_mm, s + 1)
                nc.vector.tensor_scalar_mul(
                    t_out[:, cb:cb + dw], pss[s].ap()[:, 0:dw],
                    1.0).then_inc(s_cp[s], 1)
                nc.scalar.wait_ge(s_mm, s + 1)
                nc.scalar.copy(t_out[:, cb + dw:cb + w],
                               pss[s].ap()[:, dw:w]).then_inc(s_cp[s], 1)
            elif copy_eng[s] == "dve":
                nc.vector.wait_ge(s_mm, s + 1)
                nc.vector.tensor_scalar_mul(
                    t_out[:, cb:cb + w], pss[s].ap(),
                    1.0).then_inc(s_cp[s], 1)
            else:
                nc.scalar.wait_ge(s_mm, s + 1)
                nc.scalar.copy(t_out[:, cb:cb + w],
                               pss[s].ap()).then_inc(s_cp[s], 1)

        # ---- drain: NEFF must not end before the output landed ----
        nc.sync.wait_ge(s_out, 16)
    nc.compile()
    return nc


def _get_nc():
    global _NC
    if _NC is None:
        _NC = _build()
    return _NC


def _make_in_maps(x, w, b, slices=None):
    slices = slices or SLICES
    cum = [sum(slices[:i]) for i in range(len(slices) + 1)]
    xb = np.asarray(x, dtype=np.float32).astype(BF16)
    wb = np.asarray(w, dtype=np.float32).astype(BF16)
    xbT = np.ascontiguousarray(xb.T)               # [NIN, B]
    wr = wb.reshape(KO, P, NOUT)                   # [ko, p, nout]
    xr = xbT.reshape(KO, P, B)                     # [ko, p, batch]
    in_maps = []
    for c in range(N_CORES):
        gb, gn = divmod(c, GN)
        pack = np.empty((P, INW), BF16)
        pack[:, 0:W_ELEMS] = (
            wr[:, :, gn * NS:(gn + 1) * NS].transpose(1, 0, 2).reshape(P, W_ELEMS))
        xcore = xr[:, :, gb * BS:(gb + 1) * BS]    # [ko, p, BS]
        for s in range(len(slices)):
            for k in range(KO):
                off = W_ELEMS + KO * cum[s] + k * slices[s]
                pack[:, off:off + slices[s]] = xcore[k, :, cum[s]:cum[s + 1]]
        in_maps.append({"inp": pack})
    return in_maps


def _gather(results, b):
    y = np.empty((B, NOUT), np.float32)
    for c in range(N_CORES):
        gb, gn = divmod(c, GN)
        yt = np.asarray(results[c]["y"]).reshape(NS, BS).astype(np.float32)
        y[gb * BS:(gb + 1) * BS, gn * NS:(gn + 1) * NS] = yt.T
    return y + np.asarray(b, dtype=np.float32)[None, :]


def run(x, w, b, **spmd_kwargs):
    """Run on hardware; returns (y, BassKernelResults)."""
    nc = _get_nc()
    res = run_bass_kernel_spmd(nc, _make_in_maps(x, w, b),
                               list(range(N_CORES)), **spmd_kwargs)
    return _gather(res.results, b), res


def kernel(x, w, b):
    y, _ = run(x, w, b)
    return y
